# revision 3
# baseline (speedup 1.0000x reference)
"""Trainium2 Bass kernel for nn_LocalRNN (local GRU, chunked scan).

Problem: B=32, S=2048, I=H=256, ksize=16. Each ksize-chunk runs a GRU from
h0=0, so the 32*128=4096 chunks are independent length-16 GRU chains.

Sharding: data-parallel over chunks — core c gets batch rows [4c:4c+4],
i.e. 512 chains. Weights replicated.

Per-core kernel layout ("transposed"): gate/hidden dim on partitions, chain
(seq) index on the free dim. Per step t and seq-group g (2 groups x 256 seqs):

  gates[3H, seqs] = W_ih @ x_t^T + W_hh @ h_{t-1}^T     (PSUM accumulation)
  r = sigmoid(psum_r + (b_ih+b_hh)_r)                    (ScalarE, bias port)
  z = sigmoid(psum_z + (b_ih+b_hh)_z)
  n = tanh((psum_in + b_ih_n) + r*(psum_hn + b_hh_n))    (fused DVE stt ops)
  h = n + z*(h_prev - n)

Matmul emission order per group-step: ALL x-side matmuls first, then the
h-side block (r, hn, z). The x block needs no fresh dependencies, so the PE
has a deep queue of ready work covering the other group's elementwise chain
latency; the h-side r matmuls come first in the h block because sigmoid(r)
leads the elementwise chain.

h is written straight into a per-step staging tile [128, G, 2, NS] and
DMA'd out once per step (halves output DMA count); x tiles are DMA'd once
per step covering both groups; the weights land in one DMA triggered first
so the first matmul starts as early as possible.

Matmul operands and SBUF elementwise tensors are fp16 (DVE 2x mode; values
are O(1) so fp16 range is safe); PSUM accumulation is fp32. Host
pre-transposes x / weights into DMA-friendly contiguous blocks and inverts
the output layout at the end.
"""

import sys

for _p in ("/opt/trn_rl_repo", "/root/.axon_site"):
    if _p not in sys.path:
        sys.path.insert(0, _p)

import ml_dtypes
import numpy as np

import concourse.bass as bass  # noqa: F401
import concourse.tile as tile
from concourse import bacc, mybir
from concourse.bass_utils import run_bass_kernel_spmd

# Problem constants (hardcoded per harness contract).
B, S, I, H = 32, 2048, 256, 256
KSIZE = 16
NCORES = 8
ROWS_PER_CORE = B // NCORES            # 4 batch rows per core
CHUNKS_PER_ROW = S // KSIZE            # 128
SEQS = ROWS_PER_CORE * CHUNKS_PER_ROW  # 512 chains per core
G = 2                                  # seq groups per core
NS = SEQS // G                         # 256 seqs per group
KT = 2                                 # contraction tiles (I/128 = H/128 = 2)

F32 = mybir.dt.float32
F16 = mybir.dt.float16
AF = mybir.ActivationFunctionType
OP = mybir.AluOpType

MM_DT = F16         # matmul operand + elementwise SBUF dtype
NP_MM_DT = np.float16


def build_nc():
    nc = bacc.Bacc("TRN2", target_bir_lowering=False, debug=False)

    # Inputs (host pre-transposed, contiguous per-DMA blocks).
    # wt[p, k, w, m]: w=0 -> W_ih[m, k*128+p], w=1 -> W_hh[m, k*128+p]
    w_d = nc.dram_tensor("wt", [128, KT, 2, 3 * H], MM_DT, kind="ExternalInput")
    # bias8[p, j]: j=0..3 (b_ih+b_hh)[j*128+p] (r0,r1,z0,z1);
    #              j=4,5 b_hh[2H+m*128+p]; j=6,7 b_ih[2H+m*128+p]
    bias_d = nc.dram_tensor("bias8", [128, 8], F32, kind="ExternalInput")
    # xt[t, p, g, k, s] = x_shard[seq=g*NS+s, t, i=k*128+p]
    xt_d = nc.dram_tensor("xt", [KSIZE, 128, G, KT, NS], MM_DT, kind="ExternalInput")
    # out[t, p, g, m, s] = h_t[seq=g*NS+s, hdim=m*128+p]
    out_d = nc.dram_tensor("out", [KSIZE, 128, G, 2, NS], MM_DT, kind="ExternalOutput")

    with tile.TileContext(nc) as tc:
        with (
            tc.tile_pool(name="consts", bufs=1) as consts,
            tc.tile_pool(name="xp", bufs=4) as xp,
            tc.tile_pool(name="ps", bufs=2, space="PSUM") as ps,
            tc.tile_pool(name="work", bufs=4) as work,
            tc.tile_pool(name="stp", bufs=2) as stp,
        ):
            wt = consts.tile([128, KT, 2, 3 * H], MM_DT)
            nc.sync.dma_start(wt[:], w_d.ap())
            bias = consts.tile([128, 8], F32)
            nc.sync.dma_start(bias[:], bias_d.ap())

            h_state = [None] * G
            for t in range(KSIZE):
                xs = xp.tile([128, G, KT, NS], MM_DT, tag="x")
                nc.sync.dma_start(xs[:], xt_d.ap()[t])
                stage = stp.tile([128, G, 2, NS], MM_DT, tag="st")

                for g in range(G):
                    hr = None if t == 0 else h_state[g]

                    # PSUM banks: [128, 2, NS] f32 = one 2KB bank each.
                    bank_r = ps.tile([128, 2, NS], F32, tag="r")
                    bank_z = ps.tile([128, 2, NS], F32, tag="z")
                    bank_in = ps.tile([128, 2, NS], F32, tag="in")
                    bank_hn = None if t == 0 else ps.tile([128, 2, NS], F32, tag="hn")

                    # Matmuls. Each (bank, m) accumulation group must be
                    # contiguous: a PSUM bank ("zero region") admits only one
                    # open group at a time. Group order tuned for the
                    # elementwise chain: r first (its sigmoid leads), hn next
                    # (feeds tmp), then in (pren input), z last (consumed
                    # latest, by e = z*d).
                    def mm_group(bank, m, mi, with_x, with_h):
                        col = slice(mi * 128, (mi + 1) * 128)
                        n_mm = (KT if with_x else 0) + (KT if with_h else 0)
                        i_mm = 0
                        if with_x:
                            for k in range(KT):
                                nc.tensor.matmul(
                                    bank[:, m, :], wt[:, k, 0, col], xs[:, g, k, :],
                                    start=(i_mm == 0), stop=(i_mm == n_mm - 1),
                                )
                                i_mm += 1
                        if with_h:
                            for k in range(KT):
                                nc.tensor.matmul(
                                    bank[:, m, :], wt[:, k, 1, col], hr[:, k, :],
                                    start=(i_mm == 0), stop=(i_mm == n_mm - 1),
                                )
                                i_mm += 1

                    for m in range(2):
                        mm_group(bank_r, m, m, True, t > 0)
                    if t > 0:
                        for m in range(2):
                            mm_group(bank_hn, m, 4 + m, False, True)
                    for m in range(2):
                        mm_group(bank_in, m, 4 + m, True, False)
                    for m in range(2):
                        mm_group(bank_z, m, 2 + m, True, t > 0)

                    # --- Elementwise ---
                    r_t = work.tile([128, 2, NS], MM_DT, tag="rg")
                    z_t = work.tile([128, 2, NS], MM_DT, tag="zg")
                    for mi in range(2):  # r halves first: r leads the chain
                        nc.scalar.activation(
                            r_t[:, mi, :], bank_r[:, mi, :], AF.Sigmoid,
                            bias=bias[:, mi : mi + 1],
                        )
                    for mi in range(2):  # z halves after (consumed late)
                        nc.scalar.activation(
                            z_t[:, mi, :], bank_z[:, mi, :], AF.Sigmoid,
                            bias=bias[:, 2 + mi : 3 + mi],
                        )

                    tmp = work.tile([128, 2, NS], MM_DT, tag="tmp")
                    pren = work.tile([128, 2, NS], MM_DT, tag="pren")
                    for m in range(2):
                        if t == 0:
                            # h=0: h-side n contribution is just b_hh_n.
                            nc.vector.tensor_scalar_mul(
                                tmp[:, m, :], r_t[:, m, :], bias[:, 4 + m : 5 + m]
                            )
                        else:
                            # tmp = (psum_hn + b_hh_n) * r
                            nc.vector.scalar_tensor_tensor(
                                tmp[:, m, :], bank_hn[:, m, :],
                                bias[:, 4 + m : 5 + m],
                                r_t[:, m, :], op0=OP.add, op1=OP.mult,
                            )
                        # pre_n = (psum_in + b_ih_n) + tmp
                        nc.vector.scalar_tensor_tensor(
                            pren[:, m, :], bank_in[:, m, :],
                            bias[:, 6 + m : 7 + m],
                            tmp[:, m, :], op0=OP.add, op1=OP.add,
                        )

                    n_t = work.tile([128, 2, NS], MM_DT, tag="n")
                    nc.scalar.activation(n_t[:], pren[:], AF.Tanh)

                    hnew = stage[:, g]
                    e = work.tile([128, 2, NS], MM_DT, tag="e")
                    if t == 0:
                        # h1 = n - z*n
                        nc.vector.tensor_tensor(e[:], z_t[:], n_t[:], op=OP.mult)
                        nc.vector.tensor_tensor(hnew, n_t[:], e[:], op=OP.subtract)
                    else:
                        d = work.tile([128, 2, NS], MM_DT, tag="d")
                        # h = n + z*(h_prev - n)
                        nc.vector.tensor_tensor(d[:], hr[:], n_t[:], op=OP.subtract)
                        nc.vector.tensor_tensor(e[:], z_t[:], d[:], op=OP.mult)
                        nc.vector.tensor_tensor(hnew, e[:], n_t[:], op=OP.add)

                    h_state[g] = hnew

                nc.sync.dma_start(out_d.ap()[t], stage[:])

    nc.compile()
    return nc


_NC_CACHE = None


def _get_nc():
    global _NC_CACHE
    if _NC_CACHE is None:
        _NC_CACHE = build_nc()
    return _NC_CACHE


def _prep_shared(W_ih, W_hh, b_ih, b_hh):
    # wt[p, k, w, m]
    wih_t = W_ih.T.reshape(KT, 128, 3 * H).transpose(1, 0, 2)  # [128, KT, 3H]
    whh_t = W_hh.T.reshape(KT, 128, 3 * H).transpose(1, 0, 2)
    wt = np.ascontiguousarray(
        np.stack([wih_t, whh_t], axis=2)
    ).astype(NP_MM_DT)  # [128, KT, 2, 3H]
    bsum = b_ih + b_hh
    bias8 = np.concatenate(
        [
            bsum[: 2 * H].reshape(4, 128).T,
            b_hh[2 * H :].reshape(2, 128).T,
            b_ih[2 * H :].reshape(2, 128).T,
        ],
        axis=1,
    )
    bias8 = np.ascontiguousarray(bias8).astype(np.float32)  # [128, 8]
    return wt, bias8


def _prep_core_inputs(x, shared, core):
    wt, bias8 = shared
    xc = x[core * ROWS_PER_CORE : (core + 1) * ROWS_PER_CORE]  # [4, S, I]
    xc = xc.reshape(SEQS, KSIZE, I)
    # xt[t, p, g, k, s] = xc[g*NS+s, t, k*128+p]
    xt = np.ascontiguousarray(
        xc.reshape(G, NS, KSIZE, KT, 128).transpose(2, 4, 0, 3, 1)
    ).astype(NP_MM_DT)
    return {"xt": xt, "wt": wt, "bias8": bias8}


def kernel(x, W_ih, W_hh, b_ih, b_hh, ksize):
    x = np.asarray(x, dtype=np.float32)
    W_ih = np.asarray(W_ih, dtype=np.float32)
    W_hh = np.asarray(W_hh, dtype=np.float32)
    b_ih = np.asarray(b_ih, dtype=np.float32)
    b_hh = np.asarray(b_hh, dtype=np.float32)
    assert int(ksize) == KSIZE and x.shape == (B, S, I)

    shared = _prep_shared(W_ih, W_hh, b_ih, b_hh)
    in_maps = [_prep_core_inputs(x, shared, c) for c in range(NCORES)]
    nc = _get_nc()
    res = run_bass_kernel_spmd(nc, in_maps, core_ids=list(range(NCORES)))

    out = np.empty((B, S, H), dtype=np.float32)
    for c in range(NCORES):
        oc = np.asarray(res.results[c]["out"]).astype(np.float32)  # [t,p,g,m,s]
        # h[seq=g*NS+s, t, hdim=m*128+p]
        hc = oc.transpose(2, 4, 0, 3, 1).reshape(SEQS, KSIZE, H)
        out[c * ROWS_PER_CORE : (c + 1) * ROWS_PER_CORE] = hc.reshape(
            ROWS_PER_CORE, S, H
        )
    return out


# revision 7
# speedup vs baseline: 1.0049x; 1.0049x over previous
"""Trainium2 Bass kernel for nn_LocalRNN (local GRU, chunked scan).

Problem: B=32, S=2048, I=H=256, ksize=16. Each ksize-chunk runs a GRU from
h0=0, so the 32*128=4096 chunks are independent length-16 GRU chains.

Sharding: data-parallel over chunks — core c gets batch rows [4c:4c+4],
i.e. 512 chains. Weights replicated.

Per-core kernel layout ("transposed"): gate/hidden dim on partitions, chain
(seq) index on the free dim. Per step t and seq-group g (2 groups x 256 seqs):

  gates[3H, seqs] = W_ih @ x_t^T + W_hh @ h_{t-1}^T     (PSUM accumulation)
  r = sigmoid(psum_r + (b_ih+b_hh)_r)                    (ScalarE, bias port)
  z = sigmoid(psum_z + (b_ih+b_hh)_z)
  n = tanh((psum_in + b_ih_n) + r*(psum_hn + b_hh_n))    (fused DVE stt ops)
  h = n + z*(h_prev - n)

Matmul emission order per group-step: ALL x-side matmuls first, then the
h-side block (r, hn, z). The x block needs no fresh dependencies, so the PE
has a deep queue of ready work covering the other group's elementwise chain
latency; the h-side r matmuls come first in the h block because sigmoid(r)
leads the elementwise chain.

h is written straight into a per-step staging tile [128, G, 2, NS] and
DMA'd out once per step (halves output DMA count); x tiles are DMA'd once
per step covering both groups; the weights land in one DMA triggered first
so the first matmul starts as early as possible.

Matmul operands and SBUF elementwise tensors are fp16 (DVE 2x mode; values
are O(1) so fp16 range is safe); PSUM accumulation is fp32. Host
pre-transposes x / weights into DMA-friendly contiguous blocks and inverts
the output layout at the end.
"""

import sys

for _p in ("/opt/trn_rl_repo", "/root/.axon_site"):
    if _p not in sys.path:
        sys.path.insert(0, _p)

import ml_dtypes
import numpy as np

import concourse.bass as bass  # noqa: F401
import concourse.tile as tile
from concourse import bacc, mybir
from concourse.bass_utils import run_bass_kernel_spmd

# Problem constants (hardcoded per harness contract).
B, S, I, H = 32, 2048, 256, 256
KSIZE = 16
NCORES = 8
ROWS_PER_CORE = B // NCORES            # 4 batch rows per core
CHUNKS_PER_ROW = S // KSIZE            # 128
SEQS = ROWS_PER_CORE * CHUNKS_PER_ROW  # 512 chains per core
G = 2                                  # seq groups per core
NS = SEQS // G                         # 256 seqs per group
KT = 2                                 # contraction tiles (I/128 = H/128 = 2)

F32 = mybir.dt.float32
F16 = mybir.dt.float16
AF = mybir.ActivationFunctionType
OP = mybir.AluOpType

MM_DT = F16         # matmul operand + elementwise SBUF dtype
NP_MM_DT = np.float16


def build_nc():
    nc = bacc.Bacc("TRN2", target_bir_lowering=False, debug=False)

    # Inputs (host pre-transposed, contiguous per-DMA blocks).
    # wt[p, k, w, m]: w=0 -> W_ih[m, k*128+p], w=1 -> W_hh[m, k*128+p]
    # Split along w so the W_ih half (plus x[0]) gates the first matmul
    # while W_hh (first needed ~2.6us later) streams behind it.
    w_d = nc.dram_tensor("wt", [128, KT, 2, 3 * H], MM_DT, kind="ExternalInput")
    # bias8[p, j]: j=0..3 (b_ih+b_hh)[j*128+p] (r0,r1,z0,z1);
    #              j=4,5 b_hh[2H+m*128+p]; j=6,7 b_ih[2H+m*128+p]
    bias_d = nc.dram_tensor("bias8", [128, 8], F32, kind="ExternalInput")
    # xt[t, p, g, k, s] = x_shard[seq=g*NS+s, t, i=k*128+p]
    xt_d = nc.dram_tensor("xt", [KSIZE, 128, G, KT, NS], MM_DT, kind="ExternalInput")
    # out[t, p, g, m, s] = h_t[seq=g*NS+s, hdim=m*128+p]
    out_d = nc.dram_tensor("out", [KSIZE, 128, G, 2, NS], MM_DT, kind="ExternalOutput")

    with tile.TileContext(nc) as tc:
        with (
            tc.tile_pool(name="consts", bufs=1) as consts,
            tc.tile_pool(name="xp", bufs=4) as xp,
            tc.tile_pool(name="ps", bufs=2, space="PSUM") as ps,
            tc.tile_pool(name="work", bufs=4) as work,
            tc.tile_pool(name="stp", bufs=2) as stp,
        ):
            wt = consts.tile([128, KT, 2, 3 * H], MM_DT)
            nc.sync.dma_start(wt[:, :, 0], w_d.ap()[:, :, 0])

            h_state = [None] * G
            for t in range(KSIZE):
                xs = xp.tile([128, G, KT, NS], MM_DT, tag="x")
                nc.sync.dma_start(xs[:], xt_d.ap()[t])
                if t == 0:
                    # Trigger after x[0]: W_hh and biases aren't needed until
                    # the first h-side matmul / first sigmoid.
                    nc.sync.dma_start(wt[:, :, 1], w_d.ap()[:, :, 1])
                    bias = consts.tile([128, 8], F32)
                    nc.sync.dma_start(bias[:], bias_d.ap())
                stage = stp.tile([128, G, 2, NS], MM_DT, tag="st")

                for g in range(G):
                    hr = None if t == 0 else h_state[g]

                    # PSUM banks: [128, 2, NS] f32 = one 2KB bank each.
                    bank_r = ps.tile([128, 2, NS], F32, tag="r")
                    bank_z = ps.tile([128, 2, NS], F32, tag="z")
                    bank_in = ps.tile([128, 2, NS], F32, tag="in")
                    bank_hn = None if t == 0 else ps.tile([128, 2, NS], F32, tag="hn")

                    # Matmuls. Each (bank, m) accumulation group must be
                    # contiguous: a PSUM bank ("zero region") admits only one
                    # open group at a time. Group order tuned for the
                    # elementwise chain: r first (its sigmoid leads), hn next
                    # (feeds tmp), then in (pren input), z last (consumed
                    # latest, by e = z*d).
                    def mm_group(bank, m, mi, with_x, with_h):
                        col = slice(mi * 128, (mi + 1) * 128)
                        n_mm = (KT if with_x else 0) + (KT if with_h else 0)
                        i_mm = 0
                        if with_x:
                            for k in range(KT):
                                nc.tensor.matmul(
                                    bank[:, m, :], wt[:, k, 0, col], xs[:, g, k, :],
                                    start=(i_mm == 0), stop=(i_mm == n_mm - 1),
                                )
                                i_mm += 1
                        if with_h:
                            for k in range(KT):
                                nc.tensor.matmul(
                                    bank[:, m, :], wt[:, k, 1, col], hr[:, k, :],
                                    start=(i_mm == 0), stop=(i_mm == n_mm - 1),
                                )
                                i_mm += 1

                    for m in range(2):
                        mm_group(bank_r, m, m, True, t > 0)
                    if t > 0:
                        for m in range(2):
                            mm_group(bank_hn, m, 4 + m, False, True)
                    for m in range(2):
                        mm_group(bank_in, m, 4 + m, True, False)
                    for m in range(2):
                        mm_group(bank_z, m, 2 + m, True, t > 0)

                    # --- Elementwise ---
                    r_t = work.tile([128, 2, NS], MM_DT, tag="rg")
                    z_t = work.tile([128, 2, NS], MM_DT, tag="zg")
                    for mi in range(2):  # r halves first: r leads the chain
                        nc.scalar.activation(
                            r_t[:, mi, :], bank_r[:, mi, :], AF.Sigmoid,
                            bias=bias[:, mi : mi + 1],
                        )
                    for mi in range(2):  # z halves after (consumed late)
                        nc.scalar.activation(
                            z_t[:, mi, :], bank_z[:, mi, :], AF.Sigmoid,
                            bias=bias[:, 2 + mi : 3 + mi],
                        )

                    tmp = work.tile([128, 2, NS], MM_DT, tag="tmp")
                    pren = work.tile([128, 2, NS], MM_DT, tag="pren")
                    # (GPSIMD cannot access PSUM on TRN2, so both halves
                    # stay on DVE.)
                    for m in range(2):
                        eng = nc.vector
                        if t == 0:
                            # h=0: h-side n contribution is just b_hh_n.
                            eng.tensor_scalar_mul(
                                tmp[:, m, :], r_t[:, m, :], bias[:, 4 + m : 5 + m]
                            )
                        else:
                            # tmp = (psum_hn + b_hh_n) * r
                            eng.scalar_tensor_tensor(
                                tmp[:, m, :], bank_hn[:, m, :],
                                bias[:, 4 + m : 5 + m],
                                r_t[:, m, :], op0=OP.add, op1=OP.mult,
                            )
                        # pre_n = (psum_in + b_ih_n) + tmp
                        eng.scalar_tensor_tensor(
                            pren[:, m, :], bank_in[:, m, :],
                            bias[:, 6 + m : 7 + m],
                            tmp[:, m, :], op0=OP.add, op1=OP.add,
                        )

                    n_t = work.tile([128, 2, NS], MM_DT, tag="n")
                    nc.scalar.activation(n_t[:], pren[:], AF.Tanh)

                    hnew = stage[:, g]
                    e = work.tile([128, 2, NS], MM_DT, tag="e")
                    if t == 0:
                        # h1 = n - z*n
                        nc.vector.tensor_tensor(e[:], z_t[:], n_t[:], op=OP.mult)
                        nc.vector.tensor_tensor(hnew, n_t[:], e[:], op=OP.subtract)
                    else:
                        d = work.tile([128, 2, NS], MM_DT, tag="d")
                        # h = n + z*(h_prev - n)
                        nc.vector.tensor_tensor(d[:], hr[:], n_t[:], op=OP.subtract)
                        nc.vector.tensor_tensor(e[:], z_t[:], d[:], op=OP.mult)
                        nc.vector.tensor_tensor(hnew, e[:], n_t[:], op=OP.add)

                    h_state[g] = hnew

                nc.sync.dma_start(out_d.ap()[t], stage[:])

    nc.compile()
    return nc


_NC_CACHE = None


def _get_nc():
    global _NC_CACHE
    if _NC_CACHE is None:
        _NC_CACHE = build_nc()
    return _NC_CACHE


def _prep_shared(W_ih, W_hh, b_ih, b_hh):
    # wt[p, k, w, m]
    wih_t = W_ih.T.reshape(KT, 128, 3 * H).transpose(1, 0, 2)  # [128, KT, 3H]
    whh_t = W_hh.T.reshape(KT, 128, 3 * H).transpose(1, 0, 2)
    wt = np.ascontiguousarray(
        np.stack([wih_t, whh_t], axis=2)
    ).astype(NP_MM_DT)  # [128, KT, 2, 3H]
    bsum = b_ih + b_hh
    bias8 = np.concatenate(
        [
            bsum[: 2 * H].reshape(4, 128).T,
            b_hh[2 * H :].reshape(2, 128).T,
            b_ih[2 * H :].reshape(2, 128).T,
        ],
        axis=1,
    )
    bias8 = np.ascontiguousarray(bias8).astype(np.float32)  # [128, 8]
    return wt, bias8


def _prep_core_inputs(x, shared, core):
    wt, bias8 = shared
    xc = x[core * ROWS_PER_CORE : (core + 1) * ROWS_PER_CORE]  # [4, S, I]
    xc = xc.reshape(SEQS, KSIZE, I)
    # xt[t, p, g, k, s] = xc[g*NS+s, t, k*128+p]
    xt = np.ascontiguousarray(
        xc.reshape(G, NS, KSIZE, KT, 128).transpose(2, 4, 0, 3, 1)
    ).astype(NP_MM_DT)
    return {"xt": xt, "wt": wt, "bias8": bias8}


def kernel(x, W_ih, W_hh, b_ih, b_hh, ksize):
    x = np.asarray(x, dtype=np.float32)
    W_ih = np.asarray(W_ih, dtype=np.float32)
    W_hh = np.asarray(W_hh, dtype=np.float32)
    b_ih = np.asarray(b_ih, dtype=np.float32)
    b_hh = np.asarray(b_hh, dtype=np.float32)
    assert int(ksize) == KSIZE and x.shape == (B, S, I)

    shared = _prep_shared(W_ih, W_hh, b_ih, b_hh)
    in_maps = [_prep_core_inputs(x, shared, c) for c in range(NCORES)]
    nc = _get_nc()
    res = run_bass_kernel_spmd(nc, in_maps, core_ids=list(range(NCORES)))

    out = np.empty((B, S, H), dtype=np.float32)
    for c in range(NCORES):
        oc = np.asarray(res.results[c]["out"]).astype(np.float32)  # [t,p,g,m,s]
        # h[seq=g*NS+s, t, hdim=m*128+p]
        hc = oc.transpose(2, 4, 0, 3, 1).reshape(SEQS, KSIZE, H)
        out[c * ROWS_PER_CORE : (c + 1) * ROWS_PER_CORE] = hc.reshape(
            ROWS_PER_CORE, S, H
        )
    return out


# revision 11
# speedup vs baseline: 1.0130x; 1.0081x over previous
"""Trainium2 Bass kernel for nn_LocalRNN (local GRU, chunked scan).

Problem: B=32, S=2048, I=H=256, ksize=16. Each ksize-chunk runs a GRU from
h0=0, so the 32*128=4096 chunks are independent length-16 GRU chains.

Sharding: data-parallel over chunks — core c gets batch rows [4c:4c+4],
i.e. 512 chains. Weights replicated.

Per-core kernel layout ("transposed"): gate/hidden dim on partitions, chain
(seq) index on the free dim. Per step t and seq-group g (2 groups x 256 seqs):

  gates[3H, seqs] = W_ih @ x_t^T + W_hh @ h_{t-1}^T     (PSUM accumulation)
  r = sigmoid(psum_r + (b_ih+b_hh)_r)                    (ScalarE, bias port)
  z = sigmoid(psum_z + (b_ih+b_hh)_z)
  n = tanh((psum_in + b_ih_n) + r*(psum_hn + b_hh_n))    (fused DVE stt ops)
  h = n + z*(h_prev - n)

Matmul emission order per group-step: ALL x-side matmuls first, then the
h-side block (r, hn, z). The x block needs no fresh dependencies, so the PE
has a deep queue of ready work covering the other group's elementwise chain
latency; the h-side r matmuls come first in the h block because sigmoid(r)
leads the elementwise chain.

h is written straight into a per-step staging tile [128, G, 2, NS] and
DMA'd out once per step (halves output DMA count); x tiles are DMA'd once
per step covering both groups; the weights land in one DMA triggered first
so the first matmul starts as early as possible.

Matmul operands and SBUF elementwise tensors are fp16 (DVE 2x mode; values
are O(1) so fp16 range is safe); PSUM accumulation is fp32. Host
pre-transposes x / weights into DMA-friendly contiguous blocks and inverts
the output layout at the end.
"""

import sys

for _p in ("/opt/trn_rl_repo", "/root/.axon_site"):
    if _p not in sys.path:
        sys.path.insert(0, _p)

import ml_dtypes
import numpy as np

import concourse.bass as bass  # noqa: F401
import concourse.tile as tile
from concourse import bacc, mybir
from concourse.bass_utils import run_bass_kernel_spmd

# Problem constants (hardcoded per harness contract).
B, S, I, H = 32, 2048, 256, 256
KSIZE = 16
NCORES = 8
ROWS_PER_CORE = B // NCORES            # 4 batch rows per core
CHUNKS_PER_ROW = S // KSIZE            # 128
SEQS = ROWS_PER_CORE * CHUNKS_PER_ROW  # 512 chains per core
G = 2                                  # seq groups per core
NS = SEQS // G                         # 256 seqs per group
KT = 2                                 # contraction tiles (I/128 = H/128 = 2)

F32 = mybir.dt.float32
F16 = mybir.dt.float16
AF = mybir.ActivationFunctionType
OP = mybir.AluOpType

MM_DT = F16         # matmul operand + elementwise SBUF dtype
NP_MM_DT = np.float16


def build_nc():
    nc = bacc.Bacc("TRN2", target_bir_lowering=False, debug=False)

    # Inputs (host pre-transposed, contiguous per-DMA blocks).
    # wt[p, k, w, m]: w=0 -> W_ih[m, k*128+p], w=1 -> W_hh[m, k*128+p]
    # Split along w so the W_ih half (plus x[0]) gates the first matmul
    # while W_hh (first needed ~2.6us later) streams behind it.
    w_d = nc.dram_tensor("wt", [128, KT, 2, 3 * H], MM_DT, kind="ExternalInput")
    # bias8[p, j]: j=0..3 (b_ih+b_hh)[j*128+p] (r0,r1,z0,z1);
    #              j=4,5 b_hh[2H+m*128+p]; j=6,7 b_ih[2H+m*128+p]
    bias_d = nc.dram_tensor("bias8", [128, 8], F32, kind="ExternalInput")
    # xt[t, p, g, k, s] = x_shard[seq=g*NS+s, t, i=k*128+p]
    xt_d = nc.dram_tensor("xt", [KSIZE, 128, G, KT, NS], MM_DT, kind="ExternalInput")
    # out[t, p, g, m, s] = h_t[seq=g*NS+s, hdim=m*128+p]
    out_d = nc.dram_tensor("out", [KSIZE, 128, G, 2, NS], MM_DT, kind="ExternalOutput")

    with tile.TileContext(nc) as tc:
        with (
            tc.tile_pool(name="consts", bufs=1) as consts,
            tc.tile_pool(name="xp", bufs=4) as xp,
            tc.tile_pool(name="ps", bufs=2, space="PSUM") as ps,
            tc.tile_pool(name="work", bufs=4) as work,
            tc.tile_pool(name="stp", bufs=2) as stp,
        ):
            # Stage the input DMAs so the first matmul's operands arrive
            # first: W_ih r/z columns, then x[0], then W_ih n columns
            # (needed ~0.9us into the first block), then W_hh and biases.
            wt = consts.tile([128, KT, 2, 3 * H], MM_DT)
            nc.sync.dma_start(wt[:, :, 0, : 2 * H], w_d.ap()[:, :, 0, : 2 * H])

            h_state = [None] * G
            for t in range(KSIZE):
                xs = xp.tile([128, G, KT, NS], MM_DT, tag="x")
                nc.sync.dma_start(xs[:], xt_d.ap()[t])
                if t == 0:
                    nc.sync.dma_start(
                        wt[:, :, 0, 2 * H :], w_d.ap()[:, :, 0, 2 * H :]
                    )
                    nc.sync.dma_start(wt[:, :, 1], w_d.ap()[:, :, 1])
                    bias = consts.tile([128, 8], F32)
                    nc.sync.dma_start(bias[:], bias_d.ap())
                stage = stp.tile([128, G, 2, NS], MM_DT, tag="st")

                for g in range(G):
                    hr = None if t == 0 else h_state[g]

                    # PSUM banks: [128, 2, NS] f32 = one 2KB bank each.
                    bank_r = ps.tile([128, 2, NS], F32, tag="r")
                    bank_z = ps.tile([128, 2, NS], F32, tag="z")
                    bank_in = ps.tile([128, 2, NS], F32, tag="in")
                    bank_hn = None if t == 0 else ps.tile([128, 2, NS], F32, tag="hn")

                    # Matmuls. Each (bank, m) accumulation group must be
                    # contiguous: a PSUM bank ("zero region") admits only one
                    # open group at a time. Group order tuned for the
                    # elementwise chain: r first (its sigmoid leads), hn next
                    # (feeds tmp), then in (pren input), z last (consumed
                    # latest, by e = z*d).
                    def mm_group(bank, m, mi, with_x, with_h):
                        col = slice(mi * 128, (mi + 1) * 128)
                        n_mm = (KT if with_x else 0) + (KT if with_h else 0)
                        i_mm = 0
                        if with_x:
                            for k in range(KT):
                                nc.tensor.matmul(
                                    bank[:, m, :], wt[:, k, 0, col], xs[:, g, k, :],
                                    start=(i_mm == 0), stop=(i_mm == n_mm - 1),
                                )
                                i_mm += 1
                        if with_h:
                            for k in range(KT):
                                nc.tensor.matmul(
                                    bank[:, m, :], wt[:, k, 1, col], hr[:, k, :],
                                    start=(i_mm == 0), stop=(i_mm == n_mm - 1),
                                )
                                i_mm += 1

                    for m in range(2):
                        mm_group(bank_r, m, m, True, t > 0)
                    if t > 0:
                        for m in range(2):
                            mm_group(bank_hn, m, 4 + m, False, True)
                    for m in range(2):
                        mm_group(bank_in, m, 4 + m, True, False)
                    for m in range(2):
                        mm_group(bank_z, m, 2 + m, True, t > 0)

                    # --- Elementwise ---
                    r_t = work.tile([128, 2, NS], MM_DT, tag="rg")
                    z_t = work.tile([128, 2, NS], MM_DT, tag="zg")
                    for mi in range(2):  # r halves first: r leads the chain
                        nc.scalar.activation(
                            r_t[:, mi, :], bank_r[:, mi, :], AF.Sigmoid,
                            bias=bias[:, mi : mi + 1],
                        )
                    for mi in range(2):  # z halves after (consumed late)
                        nc.scalar.activation(
                            z_t[:, mi, :], bank_z[:, mi, :], AF.Sigmoid,
                            bias=bias[:, 2 + mi : 3 + mi],
                        )

                    tmp = work.tile([128, 2, NS], MM_DT, tag="tmp")
                    pren = work.tile([128, 2, NS], MM_DT, tag="pren")
                    # (GPSIMD cannot access PSUM on TRN2, so both halves
                    # stay on DVE.)
                    for m in range(2):
                        eng = nc.vector
                        if t == 0:
                            # h=0: h-side n contribution is just b_hh_n.
                            eng.tensor_scalar_mul(
                                tmp[:, m, :], r_t[:, m, :], bias[:, 4 + m : 5 + m]
                            )
                        else:
                            # tmp = (psum_hn + b_hh_n) * r
                            eng.scalar_tensor_tensor(
                                tmp[:, m, :], bank_hn[:, m, :],
                                bias[:, 4 + m : 5 + m],
                                r_t[:, m, :], op0=OP.add, op1=OP.mult,
                            )
                        # pre_n = (psum_in + b_ih_n) + tmp
                        eng.scalar_tensor_tensor(
                            pren[:, m, :], bank_in[:, m, :],
                            bias[:, 6 + m : 7 + m],
                            tmp[:, m, :], op0=OP.add, op1=OP.add,
                        )

                    n_t = work.tile([128, 2, NS], MM_DT, tag="n")
                    nc.scalar.activation(n_t[:], pren[:], AF.Tanh)

                    hnew = stage[:, g]
                    e = work.tile([128, 2, NS], MM_DT, tag="e")
                    if t == 0:
                        # h1 = n - z*n
                        nc.vector.tensor_tensor(e[:], z_t[:], n_t[:], op=OP.mult)
                        nc.vector.tensor_tensor(hnew, n_t[:], e[:], op=OP.subtract)
                    else:
                        # h = (1-z)*n + z*h_prev. GpSimd (SBUF-only ops)
                        # precomputes zc = 1-z and w1 = z*h_prev off the
                        # critical tanh path, leaving DVE just two tail ops.
                        zc = work.tile([128, 2, NS], MM_DT, tag="zc")
                        w1 = work.tile([128, 2, NS], MM_DT, tag="w1")
                        nc.gpsimd.tensor_scalar(
                            zc[:], z_t[:], -1.0, 1.0, OP.mult, OP.add
                        )
                        nc.gpsimd.tensor_tensor(w1[:], z_t[:], hr[:], op=OP.mult)
                        nc.vector.tensor_tensor(e[:], zc[:], n_t[:], op=OP.mult)
                        nc.vector.tensor_tensor(hnew, e[:], w1[:], op=OP.add)

                    h_state[g] = hnew

                if t == KSIZE - 1:
                    # Split the last output DMA per group so g0's half drains
                    # ~3us earlier, shortening the tail.
                    for g in range(G):
                        nc.sync.dma_start(out_d.ap()[t, :, g], stage[:, g])
                else:
                    nc.sync.dma_start(out_d.ap()[t], stage[:])

    nc.compile()
    return nc


_NC_CACHE = None


def _get_nc():
    global _NC_CACHE
    if _NC_CACHE is None:
        _NC_CACHE = build_nc()
    return _NC_CACHE


def _prep_shared(W_ih, W_hh, b_ih, b_hh):
    # wt[p, k, w, m]
    wih_t = W_ih.T.reshape(KT, 128, 3 * H).transpose(1, 0, 2)  # [128, KT, 3H]
    whh_t = W_hh.T.reshape(KT, 128, 3 * H).transpose(1, 0, 2)
    wt = np.ascontiguousarray(
        np.stack([wih_t, whh_t], axis=2)
    ).astype(NP_MM_DT)  # [128, KT, 2, 3H]
    bsum = b_ih + b_hh
    bias8 = np.concatenate(
        [
            bsum[: 2 * H].reshape(4, 128).T,
            b_hh[2 * H :].reshape(2, 128).T,
            b_ih[2 * H :].reshape(2, 128).T,
        ],
        axis=1,
    )
    bias8 = np.ascontiguousarray(bias8).astype(np.float32)  # [128, 8]
    return wt, bias8


def _prep_core_inputs(x, shared, core):
    wt, bias8 = shared
    xc = x[core * ROWS_PER_CORE : (core + 1) * ROWS_PER_CORE]  # [4, S, I]
    xc = xc.reshape(SEQS, KSIZE, I)
    # xt[t, p, g, k, s] = xc[g*NS+s, t, k*128+p]
    xt = np.ascontiguousarray(
        xc.reshape(G, NS, KSIZE, KT, 128).transpose(2, 4, 0, 3, 1)
    ).astype(NP_MM_DT)
    return {"xt": xt, "wt": wt, "bias8": bias8}


def kernel(x, W_ih, W_hh, b_ih, b_hh, ksize):
    x = np.asarray(x, dtype=np.float32)
    W_ih = np.asarray(W_ih, dtype=np.float32)
    W_hh = np.asarray(W_hh, dtype=np.float32)
    b_ih = np.asarray(b_ih, dtype=np.float32)
    b_hh = np.asarray(b_hh, dtype=np.float32)
    assert int(ksize) == KSIZE and x.shape == (B, S, I)

    shared = _prep_shared(W_ih, W_hh, b_ih, b_hh)
    in_maps = [_prep_core_inputs(x, shared, c) for c in range(NCORES)]
    nc = _get_nc()
    res = run_bass_kernel_spmd(nc, in_maps, core_ids=list(range(NCORES)))

    out = np.empty((B, S, H), dtype=np.float32)
    for c in range(NCORES):
        oc = np.asarray(res.results[c]["out"]).astype(np.float32)  # [t,p,g,m,s]
        # h[seq=g*NS+s, t, hdim=m*128+p]
        hc = oc.transpose(2, 4, 0, 3, 1).reshape(SEQS, KSIZE, H)
        out[c * ROWS_PER_CORE : (c + 1) * ROWS_PER_CORE] = hc.reshape(
            ROWS_PER_CORE, S, H
        )
    return out


# revision 12
# speedup vs baseline: 1.0359x; 1.0226x over previous
"""Trainium2 Bass kernel for nn_LocalRNN (local GRU, chunked scan).

Problem: B=32, S=2048, I=H=256, ksize=16. Each ksize-chunk runs a GRU from
h0=0, so the 32*128=4096 chunks are independent length-16 GRU chains.

Sharding: data-parallel over chunks — core c gets batch rows [4c:4c+4],
i.e. 512 chains. Weights replicated.

Per-core kernel layout ("transposed"): gate/hidden dim on partitions, chain
(seq) index on the free dim. Per step t and seq-group g (2 groups x 256 seqs):

  gates[3H, seqs] = W_ih @ x_t^T + W_hh @ h_{t-1}^T     (PSUM accumulation)
  r = sigmoid(psum_r)                                    (bias pre-folded)
  z = sigmoid(psum_z)                                    (bias pre-folded)
  n = tanh((psum_in + tmp) + b_in'), tmp = r*(psum_hn + b_hh_n)
  h = (1-z)*n + z*h_prev

Bias folding: the r/z biases are folded into x on the host — x_r = x + dr
with dr = W_ir^-1 (b_ih+b_hh)_r, so W_ir @ x_r already includes the bias
and the sigmoids take no bias (one 512-elem ACT op per gate instead of two
256-elem ops with per-half bias). The z-stream x_z doubles as the n-gate
input with a host-adjusted b_in' = b_ih_n - W_in @ dz. (An n-gate delta
fold is numerically unsafe: cond(W_in) ~ 3e4.) b_in' is applied via the
tanh bias port (per-half), which lets pren be a single bias-free
tensor_tensor instead of two scalar_tensor_tensor ops.

Engine balance per group-step (t>0): PE 24 matmuls; ACT 2 sigmoids +
2 tanh halves; DVE 2 stt (tmp) + 1 tt (pren) + 2 tt (h tail); GpSimd
(SBUF-only ops) computes zc = 1-z and w1 = z*h_prev off the critical path
so the h update is h = zc*n + w1.

Matmul groups are contiguous per (bank, m) — a PSUM bank ("zero region")
admits only one open accumulation group at a time. Group order r, hn, in,
z matches the elementwise consumption order.

h is written straight into a per-step staging tile [128, G, 2, NS] and
DMA'd out once per step; x tiles are DMA'd once per step covering both
streams and groups; weight/bias DMAs are staged so the first matmul's
operands arrive first.

Matmul operands and elementwise SBUF tensors are fp16 (values are O(1);
the folded x+delta stays within ~25 so fp16 is safe); PSUM accumulation
is fp32. Host pre-transposes x / weights into DMA-friendly contiguous
blocks and inverts the output layout at the end.
"""

import sys

for _p in ("/opt/trn_rl_repo", "/root/.axon_site"):
    if _p not in sys.path:
        sys.path.insert(0, _p)

import ml_dtypes
import numpy as np

import concourse.bass as bass  # noqa: F401
import concourse.tile as tile
from concourse import bacc, mybir
from concourse.bass_utils import run_bass_kernel_spmd

# Problem constants (hardcoded per harness contract).
B, S, I, H = 32, 2048, 256, 256
KSIZE = 16
NCORES = 8
ROWS_PER_CORE = B // NCORES            # 4 batch rows per core
CHUNKS_PER_ROW = S // KSIZE            # 128
SEQS = ROWS_PER_CORE * CHUNKS_PER_ROW  # 512 chains per core
G = 2                                  # seq groups per core
NS = SEQS // G                         # 256 seqs per group
KT = 2                                 # contraction tiles (I/128 = H/128 = 2)
NV = 2                                 # x streams: v=0 -> x+dr, v=1 -> x+dz

F32 = mybir.dt.float32
F16 = mybir.dt.float16
AF = mybir.ActivationFunctionType
OP = mybir.AluOpType

MM_DT = F16         # matmul operand + elementwise SBUF dtype
NP_MM_DT = np.float16


def build_nc():
    nc = bacc.Bacc("TRN2", target_bir_lowering=False, debug=False)

    # Inputs (host pre-transposed, contiguous per-DMA blocks).
    # wt[p, k, w, m]: w=0 -> W_ih[m, k*128+p], w=1 -> W_hh[m, k*128+p]
    w_d = nc.dram_tensor("wt", [128, KT, 2, 3 * H], MM_DT, kind="ExternalInput")
    # bias4[p, j]: j=0,1 b_hh[2H+m*128+p]; j=2,3 b_in'[m*128+p]
    bias_d = nc.dram_tensor("bias4", [128, 4], F32, kind="ExternalInput")
    # xt[t, p, v, g, k, s] = (x + delta_v)[seq=g*NS+s, t, i=k*128+p]
    xt_d = nc.dram_tensor(
        "xt", [KSIZE, 128, NV, G, KT, NS], MM_DT, kind="ExternalInput"
    )
    # out[t, p, g, m, s] = h_t[seq=g*NS+s, hdim=m*128+p]
    out_d = nc.dram_tensor("out", [KSIZE, 128, G, 2, NS], MM_DT, kind="ExternalOutput")

    with tile.TileContext(nc) as tc:
        with (
            tc.tile_pool(name="consts", bufs=1) as consts,
            tc.tile_pool(name="xp", bufs=4) as xp,
            tc.tile_pool(name="ps", bufs=2, space="PSUM") as ps,
            tc.tile_pool(name="work", bufs=4) as work,
            tc.tile_pool(name="stp", bufs=2) as stp,
        ):
            # Stage input DMAs so the first matmul's operands arrive first:
            # W_ih r/z columns, then x[0] (r-stream first), then the rest.
            wt = consts.tile([128, KT, 2, 3 * H], MM_DT)
            nc.sync.dma_start(wt[:, :, 0, : 2 * H], w_d.ap()[:, :, 0, : 2 * H])

            h_state = [None] * G
            for t in range(KSIZE):
                xs = xp.tile([128, NV, G, KT, NS], MM_DT, tag="x")
                if t == 0:
                    nc.sync.dma_start(xs[:, 0], xt_d.ap()[t, :, 0])
                    nc.sync.dma_start(xs[:, 1], xt_d.ap()[t, :, 1])
                    nc.sync.dma_start(
                        wt[:, :, 0, 2 * H :], w_d.ap()[:, :, 0, 2 * H :]
                    )
                    nc.sync.dma_start(wt[:, :, 1], w_d.ap()[:, :, 1])
                    bias = consts.tile([128, 4], F32)
                    nc.sync.dma_start(bias[:], bias_d.ap())
                else:
                    nc.sync.dma_start(xs[:], xt_d.ap()[t])
                stage = stp.tile([128, G, 2, NS], MM_DT, tag="st")

                for g in range(G):
                    hr = None if t == 0 else h_state[g]

                    # PSUM banks: [128, 2, NS] f32 = one 2KB bank each.
                    bank_r = ps.tile([128, 2, NS], F32, tag="r")
                    bank_z = ps.tile([128, 2, NS], F32, tag="z")
                    bank_in = ps.tile([128, 2, NS], F32, tag="in")
                    bank_hn = None if t == 0 else ps.tile([128, 2, NS], F32, tag="hn")

                    # Matmuls. Each (bank, m) accumulation group must be
                    # contiguous: a PSUM bank ("zero region") admits only one
                    # open group at a time. Group order follows elementwise
                    # consumption: r first, then hn (feeds tmp), in (pren),
                    # z last (consumed latest, via zc/w1).
                    def mm_group(bank, m, mi, v, with_x, with_h):
                        col = slice(mi * 128, (mi + 1) * 128)
                        n_mm = (KT if with_x else 0) + (KT if with_h else 0)
                        i_mm = 0
                        if with_x:
                            for k in range(KT):
                                nc.tensor.matmul(
                                    bank[:, m, :], wt[:, k, 0, col],
                                    xs[:, v, g, k, :],
                                    start=(i_mm == 0), stop=(i_mm == n_mm - 1),
                                )
                                i_mm += 1
                        if with_h:
                            for k in range(KT):
                                nc.tensor.matmul(
                                    bank[:, m, :], wt[:, k, 1, col], hr[:, k, :],
                                    start=(i_mm == 0), stop=(i_mm == n_mm - 1),
                                )
                                i_mm += 1

                    for m in range(2):
                        mm_group(bank_r, m, m, 0, True, t > 0)
                    if t > 0:
                        for m in range(2):
                            mm_group(bank_hn, m, 4 + m, 0, False, True)
                    for m in range(2):
                        mm_group(bank_in, m, 4 + m, 1, True, False)
                    for m in range(2):
                        mm_group(bank_z, m, 2 + m, 1, True, t > 0)

                    # --- Elementwise ---
                    # Bias-free sigmoids: one 512-elem op per gate.
                    r_t = work.tile([128, 2, NS], MM_DT, tag="rg")
                    z_t = work.tile([128, 2, NS], MM_DT, tag="zg")
                    nc.scalar.activation(r_t[:], bank_r[:], AF.Sigmoid)
                    nc.scalar.activation(z_t[:], bank_z[:], AF.Sigmoid)

                    tmp = work.tile([128, 2, NS], MM_DT, tag="tmp")
                    pren = work.tile([128, 2, NS], MM_DT, tag="pren")
                    for m in range(2):
                        if t == 0:
                            # h=0: h-side n contribution is just b_hh_n.
                            nc.vector.tensor_scalar_mul(
                                tmp[:, m, :], r_t[:, m, :], bias[:, m : m + 1]
                            )
                        else:
                            # tmp = (psum_hn + b_hh_n) * r
                            nc.vector.scalar_tensor_tensor(
                                tmp[:, m, :], bank_hn[:, m, :],
                                bias[:, m : m + 1],
                                r_t[:, m, :], op0=OP.add, op1=OP.mult,
                            )
                    # pre_n = psum_in + tmp (b_in' applied via tanh bias port)
                    nc.vector.tensor_tensor(
                        pren[:], bank_in[:], tmp[:], op=OP.add
                    )

                    n_t = work.tile([128, 2, NS], MM_DT, tag="n")
                    for m in range(2):
                        nc.scalar.activation(
                            n_t[:, m, :], pren[:, m, :], AF.Tanh,
                            bias=bias[:, 2 + m : 3 + m],
                        )

                    hnew = stage[:, g]
                    e = work.tile([128, 2, NS], MM_DT, tag="e")
                    if t == 0:
                        # h1 = n - z*n
                        nc.vector.tensor_tensor(e[:], z_t[:], n_t[:], op=OP.mult)
                        nc.vector.tensor_tensor(hnew, n_t[:], e[:], op=OP.subtract)
                    else:
                        # h = (1-z)*n + z*h_prev. GpSimd (SBUF-only ops)
                        # precomputes zc = 1-z and w1 = z*h_prev off the
                        # critical tanh path, leaving DVE just two tail ops.
                        zc = work.tile([128, 2, NS], MM_DT, tag="zc")
                        w1 = work.tile([128, 2, NS], MM_DT, tag="w1")
                        nc.gpsimd.tensor_scalar(
                            zc[:], z_t[:], -1.0, 1.0, OP.mult, OP.add
                        )
                        nc.gpsimd.tensor_tensor(w1[:], z_t[:], hr[:], op=OP.mult)
                        nc.vector.tensor_tensor(e[:], zc[:], n_t[:], op=OP.mult)
                        nc.vector.tensor_tensor(hnew, e[:], w1[:], op=OP.add)

                    h_state[g] = hnew

                if t == KSIZE - 1:
                    # Split the last output DMA per group so g0's half drains
                    # ~3us earlier, shortening the tail.
                    for g in range(G):
                        nc.sync.dma_start(out_d.ap()[t, :, g], stage[:, g])
                else:
                    nc.sync.dma_start(out_d.ap()[t], stage[:])

    nc.compile()
    return nc


_NC_CACHE = None


def _get_nc():
    global _NC_CACHE
    if _NC_CACHE is None:
        _NC_CACHE = build_nc()
    return _NC_CACHE


def _prep_shared(W_ih, W_hh, b_ih, b_hh):
    # wt[p, k, w, m]
    wih_t = W_ih.T.reshape(KT, 128, 3 * H).transpose(1, 0, 2)  # [128, KT, 3H]
    whh_t = W_hh.T.reshape(KT, 128, 3 * H).transpose(1, 0, 2)
    wt = np.ascontiguousarray(
        np.stack([wih_t, whh_t], axis=2)
    ).astype(NP_MM_DT)  # [128, KT, 2, 3H]

    # Bias folding deltas (float64 for the solves).
    W64 = W_ih.astype(np.float64)
    bsum = (b_ih + b_hh).astype(np.float64)
    dr = np.linalg.solve(W64[:H], bsum[:H])
    dz = np.linalg.solve(W64[H : 2 * H], bsum[H : 2 * H])
    # n-gate rides the z-stream: b_in' = b_ih_n - W_in @ dz
    bin_adj = b_ih[2 * H :].astype(np.float64) - W64[2 * H :] @ dz

    bias4 = np.concatenate(
        [
            b_hh[2 * H :].reshape(2, 128).T,
            bin_adj.reshape(2, 128).T,
        ],
        axis=1,
    )
    bias4 = np.ascontiguousarray(bias4).astype(np.float32)  # [128, 4]
    return wt, bias4, dr, dz


def _prep_core_inputs(x, shared, core):
    wt, bias4, dr, dz = shared
    xc = x[core * ROWS_PER_CORE : (core + 1) * ROWS_PER_CORE]  # [4, S, I]
    xc = xc.reshape(SEQS, KSIZE, I).astype(np.float64)
    # xt[t, p, v, g, k, s] = (xc + delta_v)[g*NS+s, t, k*128+p]
    xv = np.stack([xc + dr, xc + dz], axis=0)  # [NV, SEQS, K, I]
    xt = np.ascontiguousarray(
        xv.reshape(NV, G, NS, KSIZE, KT, 128).transpose(3, 5, 0, 1, 4, 2)
    ).astype(NP_MM_DT)
    return {"xt": xt, "wt": wt, "bias4": bias4}


def kernel(x, W_ih, W_hh, b_ih, b_hh, ksize):
    x = np.asarray(x, dtype=np.float32)
    W_ih = np.asarray(W_ih, dtype=np.float32)
    W_hh = np.asarray(W_hh, dtype=np.float32)
    b_ih = np.asarray(b_ih, dtype=np.float32)
    b_hh = np.asarray(b_hh, dtype=np.float32)
    assert int(ksize) == KSIZE and x.shape == (B, S, I)

    shared = _prep_shared(W_ih, W_hh, b_ih, b_hh)
    in_maps = [_prep_core_inputs(x, shared, c) for c in range(NCORES)]
    nc = _get_nc()
    res = run_bass_kernel_spmd(nc, in_maps, core_ids=list(range(NCORES)))

    out = np.empty((B, S, H), dtype=np.float32)
    for c in range(NCORES):
        oc = np.asarray(res.results[c]["out"]).astype(np.float32)  # [t,p,g,m,s]
        # h[seq=g*NS+s, t, hdim=m*128+p]
        hc = oc.transpose(2, 4, 0, 3, 1).reshape(SEQS, KSIZE, H)
        out[c * ROWS_PER_CORE : (c + 1) * ROWS_PER_CORE] = hc.reshape(
            ROWS_PER_CORE, S, H
        )
    return out
